# revision 4
# baseline (speedup 1.0000x reference)
"""Masked multi-head attention (fused QKV) on 8 trn2 NeuronCores.

Problem (full shapes): x [2, 2048, 1024] f32, W [3072, 1024], b [3072].
  z = x @ W.T + b ; k,q,v = split(z) ; heads H=16, hd=64
  out = softmax(causal(q k^T / sqrt(1024))) v   -> [2, 2048, 1024]

Sharding: core c handles batch n=c//4 and head group g=c%4 (4 heads).
Each core is fully independent (data + head parallel, no collectives).

Per-core device program (v2 of the fp8-DoubleRow kernel):
  0) Projections run as residual-fp8 DoubleRow: x = x_hi + x_lo and
     16*W = W_hi + W_lo in e4m3 (host-side), z = xh@Wh + xh@Wl + xl@Wh
     accumulated in f32 PSUM.  Host packs hi/lo as one [D, 2, *] tensor
     per input so big blocks move in one DMA each.
  1) kq weights are hp-major ([k_hp0|q_hp0|k_hp1|q_hp1]) so the startup
     critical path only loads the hp0 half (256 cols) before the first
     projection; DMA order is cost-model-driven (the 16-engine DMA pool
     serializes transfers at ~360 B/ns and each completion semaphore
     takes ~900ns to propagate).
  2) Scores per (q-block 512, head pair) unit: 2 fp8 DoubleRow matmuls
     per k-tile into a 2-bank PSUM tile, one ACT exp (the global pacer)
     evacuates both heads to bf16 pt.  Unit order
     (0,0),(0,1),(1,0),(2,0),(3,0),(3,1),(2,1),(1,1) starts on the two
     smallest units (whose projections are resident earliest) and ends
     on a mid-size unit whose own PV chains drain history-first.
  3) Filler work (deferred projections, the previous unit's PV chains)
     is placed by a greedy scheduler that simulates ACT/PE prefix times
     with the instruction cost model and only inserts a filler where it
     cannot stall the exp stream; per-filler DMA-readiness estimates
     and deadline slots (for next unit's diag dependencies) constrain
     placement.
  4) PV in natural layout, f32 PSUM [128, 65] per (head, q-tile); DVE
     reciprocal of the fused denominator column + tensor_scalar_mul.
     Late units stream per-q-tile output DMAs so the tail is short.

Numerics identical to v1: bf16 projection ~2e-4; fp8 q/k quantization
dominates at rel_err ~1.3e-2 (harness gate 2e-2).

_split_matmul_waits() is a required legalization for this compiler
build: every engine instruction may carry at most one semaphore wait.
"""

import numpy as np

import concourse.bass as bass
import concourse.mybir as mybir
import concourse.tile as tile
from concourse.bass_utils import run_bass_kernel_spmd

F32 = mybir.dt.float32
BF = mybir.dt.bfloat16
F8 = mybir.dt.float8e4

FP8_SCORES = True

N, S, D = 2, 2048, 1024
H, HD = 16, 64
P = 128
QB = 512                 # q block (free dim per matmul)
NQB = S // QB            # 4
NKT = S // P             # 16 k tiles
ND = D // P              # 8 contraction tiles
NHC = 4                  # heads per core
EKQ = 2 * NHC * HD       # 512 = k+q rows per core
EV = NHC * HD            # 256 = v rows per core
WS = 16.0                # host W/b pre-scale (fp8 range)
SCALE = 1.0 / 32.0 / (WS * WS)   # 1/sqrt(1024), W-scale compensated

AF = mybir.ActivationFunctionType
ALU = mybir.AluOpType

# ---- cost-model constants for the greedy filler scheduler ----
ACT_C = 0.8333           # ns/elem on ACT
PE_C = 0.4167            # ns/col full rate
SEM = 160.0              # cross-engine handoff guard
GUARD = 120.0


def _exp_ns(w):
    return 2 * w * ACT_C + 185.0


def _mm_tile_ns(w, direct):
    # two matmuls (one per interleaved head) per k-tile
    if not FP8_SCORES:
        return 2 * w * PE_C
    return (2 * w * PE_C) if direct else (w * PE_C)


def _split_matmul_waits(nc):
    """Move extra semaphore waits onto preceding same-engine NOPs.

    The walrus codegen for self-loading matmuls folds waits into the
    LDWEIGHTS struct, which has room for a single sync-wait command;
    sequencer NOPs on the same engine execute in program order, so
    hoisting each wait onto its own NOP is semantics-preserving.
    """
    import bass_rust

    moved = 0
    for bb in nc.main_func.blocks:
        out = []
        for ins in bb.instructions:
            si = ins.sync_info
            keep = 0 if isinstance(ins, bass_rust.InstMatmult) else 1
            if (
                not isinstance(ins, bass_rust.InstNoOp)
                and si is not None
                and len(si.on_wait) > keep
            ):
                hoist = si.on_wait[keep:] if keep else si.on_wait
                for j, w in enumerate(hoist):
                    out.append(
                        bass_rust.InstNoOp(
                            name=f"{ins.name}-hw{j}",
                            engine=ins.engine,
                            sync_info=mybir.SyncInfo(on_wait=[w], on_update=[]),
                        )
                    )
                    moved += 1
                ins.sync_info = mybir.SyncInfo(
                    on_wait=list(si.on_wait[:keep]), on_update=list(si.on_update)
                )
            out.append(ins)
        bb.instructions[:] = out
    return moved


def build_nc(split_waits=True):
    nc = bass.Bass()

    # packed hi/lo inputs: [..., 2, ...] u-plane (0=hi, 1=lo)
    xPK = nc.dram_tensor("xPK", [D, 2, S], F8, kind="ExternalInput")
    wkq = nc.dram_tensor("wkq", [D, 2, EKQ], F8, kind="ExternalInput")
    wv = nc.dram_tensor("wv", [D, 2, EV], F8, kind="ExternalInput")
    bkq = nc.dram_tensor("bkq", [P, 4], F32, kind="ExternalInput")
    bv = nc.dram_tensor("bv", [1, EV], F32, kind="ExternalInput")
    o = nc.dram_tensor("o", [S, EV], F32, kind="ExternalOutput")

    x_v = xPK.rearrange("(dt p) u s -> p u dt s", p=P)     # [128, 2, 8, 2048]
    wkq_v = wkq.rearrange("(dt p) u e -> p u dt e", p=P)   # [128, 2, 8, 512]
    wv_v = wv.rearrange("(dt p) u e -> p u dt e", p=P)     # [128, 2, 8, 256]
    o_v = o.rearrange("(qt p) c -> p qt c", p=P)           # [128, 16, 256]

    with tile.TileContext(nc) as tc:
        with (
            tc.tile_pool(name="const", bufs=1) as const,
            tc.tile_pool(name="big", bufs=1) as big,
            tc.tile_pool(name="xpool", bufs=2) as xpool,
            tc.tile_pool(name="work", bufs=2) as work,
            tc.tile_pool(name="opool", bufs=2) as opool,
            tc.tile_pool(name="proj_ps", bufs=2, space="PSUM") as proj_ps,
            tc.tile_pool(name="st_ps", bufs=2, space="PSUM") as st_ps,
            tc.tile_pool(name="pv_ps", bufs=2, space="PSUM") as pv_ps,
        ):
            # ---- constants ----
            onesb = const.tile([P, 1], BF)
            nc.vector.memset(onesb, 1.0)
            wsb = const.tile([P, 1], BF)
            nc.vector.memset(wsb, WS)
            # warm the ACT exp table while DMAs run
            dummy = const.tile([1, 2], F32)
            nc.gpsimd.memset(dummy, 0.0)
            nc.scalar.activation(dummy, dummy, AF.Exp)
            # diagonal causal mask for the odd head: mask[p, j] = (j >= p)
            mask_sb = const.tile([P, QB], BF)
            nc.gpsimd.affine_select(
                out=mask_sb,
                in_=onesb.to_broadcast((P, QB)),
                compare_op=ALU.is_ge, fill=0.0,
                base=0, channel_multiplier=-1,
                pattern=[[1, QB]],
            )

            # ---- input DMAs, critical-path first ----
            # wkq is hp-major: cols [0:256] = (k,q) of head pair 0.
            # Pass order is (hi*hi, hi*lo, lo*hi), so the last DMA the
            # first exp waits on is xq0-lo; wkql-p0 (small) lands before.
            wkq_sb = const.tile([P, 2, ND, EKQ], F8)
            wv_sb = const.tile([P, 2, ND, EV], F8)
            bkq_sb = const.tile([P, 4], F32)
            bvb = const.tile([P, EV], F32)
            xqbs = []
            for qb in range(NQB):
                xq = xpool.tile([P, 2, ND, QB], F8, tag=f"xqb{qb}", bufs=1,
                                name=f"xqb{qb}")
                xqbs.append(xq)

            t0 = 1.06e3  # first transfer start (after first HWDGE gen)
            cum = [t0]

            def dma(out_ap, in_ap, kbytes):
                nc.sync.dma_start(out_ap, in_ap)
                cum[0] += kbytes * 1024.0 / 360.0
                return cum[0] + 950.0  # sem propagation

            r_wkqh0 = dma(wkq_sb[:, 0, :, 0:256], wkq_v[:, 0, :, 0:256], 256)
            r_xq0h = dma(xqbs[0][:, 0], x_v[:, 0, :, 0:QB], 512)
            r_wkql0 = dma(wkq_sb[:, 1, :, 0:256], wkq_v[:, 1, :, 0:256], 256)
            r_xq0l = dma(xqbs[0][:, 1], x_v[:, 1, :, 0:QB], 512)
            r_bkq = dma(bkq_sb, bkq[:, :], 2)
            r_wkq1 = dma(wkq_sb[:, :, :, 256:512], wkq_v[:, :, :, 256:512], 512)
            r_wv = dma(wv_sb, wv_v[:, :, :, :], 512)
            r_bv = dma(bvb, bv[:, :].partition_broadcast(P), 1)
            r_xq1h = dma(xqbs[1][:, 0], x_v[:, 0, :, QB:2 * QB], 512)
            r_xq1l = dma(xqbs[1][:, 1], x_v[:, 1, :, QB:2 * QB], 512)
            r_xq2 = dma(xqbs[2][:, :], x_v[:, :, :, 2 * QB:3 * QB], 1024)
            r_xq3 = dma(xqbs[3][:, :], x_v[:, :, :, 3 * QB:4 * QB], 1024)
            x_ready = [(r_xq0h, r_xq0l), (r_xq1h, r_xq1l),
                       (r_xq2, r_xq2), (r_xq3, r_xq3)]
            wkq_ready = [(r_wkqh0, r_wkql0), (r_wkq1, r_wkq1)]

            # ---- persistent state ----
            # zkq [p, kq, hp, s]; e-tile t -> (kq=t%2, hp=t//2)
            if FP8_SCORES:
                zkq = big.tile([P, 2, 2, S], F8)
                # DoubleRow re-layout [hl*32+p, kq, hp, g, s], hd = g*32+p
                kq8 = big.tile([HD, 2, 2, 2, S], F8)
            else:
                zkq = big.tile([P, 4, S], BF)
            # v natural + WS column: [p, ktile, head, 65]
            vsb = big.tile([P, NKT, NHC, HD + 1], BF)
            nc.vector.tensor_copy(
                vsb[:, :, :, HD:HD + 1],
                wsb[:, :, None].to_broadcast((P, NKT, NHC, 1)),
            )
            # exp(S^T) per unit: [p, head, ktile, q], double-buffered
            pta = big.tile([P, 2, NKT, QB], BF)
            ptb = big.tile([P, 2, NKT, QB], BF)
            pts = [pta, ptb]

            proj_state = {}
            DR = mybir.MatmulPerfMode.DoubleRow
            # residual passes: z = xh@Wh + xh@Wl + xl@Wh
            # order (hi*hi, hi*lo, lo*hi): the lo-x pass is last, so the
            # critical DMA chain ends on the small wkql-p0 block.
            PASSES = ((0, 0), (0, 1), (1, 0))

            def proj_kq_half(qb, t, half, evac_act=False):
                if half == 0:
                    pzp = proj_ps.tile([P, QB], F32, tag="projps",
                                       name=f"pzp{qb}_{t}")
                    proj_state[(qb, t)] = pzp
                else:
                    pzp = proj_state.pop((qb, t)) if half == 2 \
                        else proj_state[(qb, t)]
                xi, wi = PASSES[half]
                for p2 in range(4):
                    nc.tensor.matmul(
                        pzp,
                        lhsT=wkq_sb[:, wi, 2 * p2:2 * p2 + 2,
                                    t * P:(t + 1) * P],
                        rhs=xqbs[qb][:, xi, 2 * p2:2 * p2 + 2, :],
                        start=(half == 0 and p2 == 0),
                        stop=(half == 2 and p2 == 3),
                        perf_mode=DR,
                    )
                if half < 2:
                    return
                if FP8_SCORES:
                    out = zkq[:, t % 2, t // 2, qb * QB:(qb + 1) * QB]
                else:
                    out = zkq[:, t, qb * QB:(qb + 1) * QB]
                if evac_act:
                    nc.scalar.activation(
                        out, pzp, AF.Identity, bias=bkq_sb[:, t:t + 1]
                    )
                else:
                    nc.vector.tensor_scalar_add(out, pzp, bkq_sb[:, t:t + 1])
                if FP8_SCORES and t % 2 == 1:
                    # both e-tiles of head pair hp = t//2 evacuated:
                    # DoubleRow re-layout via SBUF->SBUF DMAs
                    hp = t // 2
                    qw = slice(qb * QB, (qb + 1) * QB)
                    for hl in range(2):
                        for g in range(2):
                            nc.sync.dma_start(
                                kq8[hl * 32:(hl + 1) * 32, :, hp, g, qw],
                                zkq[hl * HD + g * 32:hl * HD + g * 32 + 32,
                                    :, hp, qw],
                            )

            def proj_v1(qb, qt4):
                qt = qb * 4 + qt4
                pvp = proj_ps.tile([P, QB], F32, tag="projps",
                                   name=f"pvp{qt}")
                for hi in range(3):
                    xi, wi = PASSES[hi]
                    for p2 in range(4):
                        nc.tensor.matmul(
                            pvp[:, :EV],
                            lhsT=xqbs[qb][:, xi, 2 * p2:2 * p2 + 2,
                                          qt4 * P:(qt4 + 1) * P],
                            rhs=wv_sb[:, wi, 2 * p2:2 * p2 + 2, :],
                            start=(hi == 0 and p2 == 0),
                            stop=(hi == 2 and p2 == 3),
                            perf_mode=DR,
                        )
                nc.vector.tensor_tensor(
                    vsb[:, qt, :, 0:HD],
                    pvp[:, :EV].rearrange("p (h d) -> p h d", d=HD),
                    bvb.rearrange("p (h d) -> p h d", d=HD),
                    mybir.AluOpType.add,
                )

            def pv_chunk(qb, hp, qt4, hl, osb, pt, hist_first=False,
                         dma_qt=False):
                nkt_q = 4 * qb + qt4 + 1
                if hist_first:
                    kts = list(range(nkt_q))
                else:
                    kts = list(range(4 * qb, nkt_q)) + list(range(0, 4 * qb))
                pvo = pv_ps.tile([P, HD + 1], F32, tag="pv", name="pvo")
                for i, kt in enumerate(kts):
                    nc.tensor.matmul(
                        pvo,
                        lhsT=pt[:, hl, kt, qt4 * P:(qt4 + 1) * P],
                        rhs=vsb[:, kt, 2 * hp + hl, :],
                        start=(i == 0), stop=(i == nkt_q - 1),
                    )
                h = 2 * hp + hl
                rs = work.tile([P, 1], F32, tag="rs", bufs=4)
                nc.vector.reciprocal(rs, pvo[:, HD:HD + 1])
                nc.vector.tensor_scalar_mul(
                    osb[:, qt4, HD * h:HD * (h + 1)],
                    pvo[:, 0:HD], rs,
                )
                if dma_qt and hl == 1:
                    nc.sync.dma_start(
                        o_v[:, qb * 4 + qt4, hp * P:(hp + 1) * P],
                        osb[:, qt4, hp * P:(hp + 1) * P],
                    )

            # ---- greedy filler scheduler state ----
            sched = {"pe": 0.0, "act": 0.0}

            def attn_scores(qb, hp, pt, groups, direct=False,
                            hist_first=False):
                if hist_first:
                    kts = list(range(0, 4 * qb + 4))
                else:
                    kts = list(range(4 * qb, 4 * qb + 4)) + \
                        list(range(4 * qb))
                n = len(kts)
                gq = [list(g) for g in groups]
                exp_end = {}

                def emit(f):
                    f["fn"]()
                    sched["pe"] = max(sched["pe"], f.get("ready", 0.0)) \
                        + f["pe"]

                for i, kt in enumerate(kts):
                    r = kt - 4 * qb
                    diag = 0 <= r < 4
                    off = P * r if diag else 0
                    w = QB - off
                    mmc = _mm_tile_ns(w, direct and diag)
                    # forced fillers (deadline reached)
                    for g in gq:
                        while g and g[0].get("dl", 10 ** 9) <= i:
                            emit(g.pop(0))
                    # opportunistic fillers while ACT has slack
                    progress = True
                    while progress:
                        progress = False
                        for g in gq:
                            if (g and g[0].get("es", -1) <= i
                                    and g[0].get("ready", 0.0) <= sched["pe"]
                                    and sched["pe"] + g[0]["pe"] + mmc
                                    + GUARD <= sched["act"]):
                                emit(g.pop(0))
                                progress = True
                    # st ring WAR: mm_i waits exp_{i-2}
                    if i >= 2:
                        sched["pe"] = max(sched["pe"],
                                          exp_end[i - 2] + 100.0)
                    stp = st_ps.tile([P, 2, QB], F32, tag="st")
                    for hl in range(2):
                        if FP8_SCORES and direct and diag:
                            base = HD * hl
                            nc.tensor.matmul(
                                stp[:, hl, off:QB],
                                lhsT=zkq[base:base + HD, 0, hp,
                                         kt * P:(kt + 1) * P],
                                rhs=zkq[base:base + HD, 1, hp,
                                        qb * QB + off:(qb + 1) * QB],
                                start=True, stop=True,
                            )
                        elif FP8_SCORES:
                            nc.tensor.matmul(
                                stp[:, hl, off:QB],
                                lhsT=kq8[hl * 32:(hl + 1) * 32, 0, hp, :,
                                         kt * P:(kt + 1) * P],
                                rhs=kq8[hl * 32:(hl + 1) * 32, 1, hp, :,
                                        qb * QB + off:(qb + 1) * QB],
                                start=True, stop=True,
                                perf_mode=DR,
                            )
                        else:
                            base = HD * hl
                            nc.tensor.matmul(
                                stp[:, hl, off:QB],
                                lhsT=zkq[base:base + HD, 2 * hp,
                                         kt * P:(kt + 1) * P],
                                rhs=zkq[base:base + HD, 2 * hp + 1,
                                        qb * QB + off:(qb + 1) * QB],
                                start=True, stop=True,
                            )
                    sched["pe"] += mmc
                    est = max(sched["act"], sched["pe"] + SEM)
                    nc.scalar.activation(
                        pt[:, :, kt, off:QB],
                        stp[:, :, off:QB],
                        AF.Exp, scale=SCALE,
                    )
                    sched["act"] = est + _exp_ns(w)
                    exp_end[i] = sched["act"]
                    if diag:
                        nc.gpsimd.affine_select(
                            out=pt[:, 0, kt, off:QB],
                            in_=pt[:, 0, kt, off:QB],
                            compare_op=ALU.is_ge, fill=0.0,
                            base=0, channel_multiplier=-1,
                            pattern=[[1, w]],
                        )
                        nc.vector.tensor_mul(
                            out=pt[:, 1, kt, off:QB],
                            in0=pt[:, 1, kt, off:QB],
                            in1=mask_sb[:, 0:w],
                        )
                # end-of-unit drain
                for g in gq:
                    while g:
                        emit(g.pop(0))

            # ---- filler group constructors ----
            def K(qb2, pair, dl=None):
                tiles = (2 * pair, 2 * pair + 1)
                out = []
                for t in tiles:
                    for h in range(3):
                        xi, wi = PASSES[h]
                        ready = max(x_ready[qb2][xi], wkq_ready[pair][wi])
                        f = {
                            "fn": (lambda t=t, h=h, q=qb2:
                                   proj_kq_half(q, t, h)),
                            "pe": 427.0, "ready": ready,
                        }
                        if dl is not None:
                            f["dl"] = dl
                        out.append(f)
                return out

            def V(qb2):
                ready = max(x_ready[qb2][1], r_wv, r_bv)
                return [
                    {"fn": (lambda q4=q4, q=qb2: proj_v1(q, q4)),
                     "pe": 640.0, "ready": ready}
                    for q4 in range(4)
                ]

            def PV(pqb, php, ppt, v_qb=None, dma_qt=False):
                # previous unit's PV chains; optionally interleave the
                # V-projection whose vsb tiles those chains read
                out = []
                vg = V(v_qb) if v_qb is not None else []
                for q4 in range(4):
                    if vg:
                        out.append(vg[q4])
                    for hl in range(2):
                        out.append({
                            "fn": (lambda q4=q4, hl=hl:
                                   pv_chunk(pqb, php, q4, hl, osbs[pqb],
                                            ppt, dma_qt=dma_qt)),
                            "pe": (4 * pqb + q4 + 1) * 27.0 + 100.0,
                        })
                return out

            # ---- prologue: pass-major proj of (k,q) head pair 0, qb0 ----
            for half in range(3):
                for t in (0, 1):
                    proj_kq_half(0, t, half, evac_act=(t == 1))

            # ---- unit schedule ----
            units = [(0, 0), (0, 1), (1, 0), (2, 0),
                     (3, 0), (3, 1), (2, 1), (1, 1)]
            osbs = {}
            prev = None
            sched["pe"] = 8100.0
            sched["act"] = 8300.0
            for ui, (qb, hp) in enumerate(units):
                pt = pts[ui % 2]
                last = ui == len(units) - 1
                if qb not in osbs:
                    osbs[qb] = opool.tile([P, 4, EV], F32, tag="osb",
                                          bufs=4, name=f"osb{qb}")
                n_slots = 4 * qb + 4
                groups = []
                if prev is not None:
                    pqb, php, ppt = prev
                    # V(j) interleaves with the first PV chains reading it
                    v_qb = {1: 0, 3: 1, 4: 2, 5: 3}.get(ui)
                    groups.append(PV(pqb, php, ppt, v_qb=v_qb,
                                     dma_qt=(ui == 7)))
                if (qb, hp) == (0, 0):
                    groups.append(K(0, 1))
                elif (qb, hp) == (0, 1):
                    groups.append(K(1, 0))
                elif (qb, hp) == (1, 0):
                    groups.append(K(2, 0, dl=n_slots - 2))
                elif (qb, hp) == (2, 0):
                    groups.append(K(3, 0, dl=n_slots - 2))
                    groups.append(K(1, 1))
                elif (qb, hp) == (3, 0):
                    groups.append(K(3, 1, dl=n_slots - 2))
                    groups.append(K(2, 1))
                if last:
                    own = []
                    for q4 in range(4):
                        for hl in range(2):
                            own.append({
                                "fn": (lambda q4=q4, hl=hl:
                                       pv_chunk(qb, hp, q4, hl, osbs[qb],
                                                pt, hist_first=True,
                                                dma_qt=True)),
                                "pe": (4 * qb + q4 + 1) * 27.0 + 100.0,
                                "es": 4 * qb + q4 + 1,
                            })
                    groups.append(own)
                attn_scores(qb, hp, pt, groups,
                            direct=(qb == 0 or (qb, hp) == (1, 0)),
                            hist_first=last)
                if prev is not None and ui != 7:
                    # previous unit's half-row is complete
                    nc.sync.dma_start(
                        o_v[:, pqb * 4:(pqb + 1) * 4,
                            php * P:(php + 1) * P],
                        osbs[pqb][:, :, php * P:(php + 1) * P],
                    )
                prev = (qb, hp, pt)

    if split_waits:
        _split_matmul_waits(nc)
    return nc


_nc_cache = None


def _get_nc():
    global _nc_cache
    if _nc_cache is None:
        _nc_cache = build_nc()
    return _nc_cache


def make_in_maps(x, W, b):
    import ml_dtypes

    f8 = ml_dtypes.float8_e4m3

    def hilo_pk(a):
        hi = a.astype(f8)
        lo = (a - hi.astype(np.float32)).astype(f8)
        return np.ascontiguousarray(np.stack([hi, lo], axis=1))

    x = np.asarray(x, dtype=np.float32)
    W = np.asarray(W, dtype=np.float32)
    b = np.asarray(b, dtype=np.float32)
    in_maps = []
    xPKs = [hilo_pk(x[n].T) for n in range(N)]
    for c in range(8):
        n, g = divmod(c, 4)
        rk = slice(256 * g, 256 * g + 256)
        rq = slice(D + 256 * g, D + 256 * g + 256)
        rv = slice(2 * D + 256 * g, 2 * D + 256 * g + 256)
        Wk, Wq, Wv = W[rk], W[rq], W[rv]
        # hp-major column order: [k_hp0 | q_hp0 | k_hp1 | q_hp1]
        wkq_cols = np.concatenate(
            [Wk[:128], Wq[:128], Wk[128:], Wq[128:]], axis=0
        ).T * 16.0
        bk, bq = b[rk], b[rq]
        bkq_cols = np.concatenate(
            [bk[:128], bq[:128], bk[128:], bq[128:]]
        ).reshape(4, P).T * 16.0
        in_maps.append({
            "xPK": xPKs[n],
            "wkq": hilo_pk(wkq_cols),
            "wv": hilo_pk(Wv.T * 16.0),
            "bkq": np.ascontiguousarray(bkq_cols),
            "bv": np.ascontiguousarray(b[rv].reshape(1, EV) * 16.0),
        })
    return in_maps


def run(inputs, **kwargs):
    nc = _get_nc()
    in_maps = make_in_maps(inputs["x"], inputs["W"], inputs["b"])
    res = run_bass_kernel_spmd(nc, in_maps, core_ids=list(range(8)), **kwargs)
    out = np.empty((N, S, D), dtype=np.float32)
    for c in range(8):
        n, g = divmod(c, 4)
        out[n, :, 256 * g:256 * g + 256] = res.results[c]["o"]
    return out, res


def kernel(**inputs):
    out, _ = run(inputs)
    return out


# revision 11
# speedup vs baseline: 1.0419x; 1.0419x over previous
"""Masked multi-head attention (fused QKV) on 8 trn2 NeuronCores.

Problem (full shapes): x [2, 2048, 1024] f32, W [3072, 1024], b [3072].
  z = x @ W.T + b ; k,q,v = split(z) ; heads H=16, hd=64
  out = softmax(causal(q k^T / sqrt(1024))) v   -> [2, 2048, 1024]

Sharding: core c handles batch n=c//4 and head group g=c%4 (4 heads).
Each core is fully independent (data + head parallel, no collectives).

Per-core device program (v2 of the fp8-DoubleRow kernel):
  0) Projections run as residual-fp8 DoubleRow: x = x_hi + x_lo and
     16*W = W_hi + W_lo in e4m3 (host-side), z = xh@Wh + xh@Wl + xl@Wh
     accumulated in f32 PSUM.  Host packs hi/lo as one [D, 2, *] tensor
     per input so big blocks move in one DMA each.
  1) kq weights are hp-major ([k_hp0|q_hp0|k_hp1|q_hp1]) so the startup
     critical path only loads the hp0 half (256 cols) before the first
     projection; DMA order is cost-model-driven (the 16-engine DMA pool
     serializes transfers at ~360 B/ns and each completion semaphore
     takes ~900ns to propagate).
  2) Scores per (q-block 512, head pair) unit: 2 fp8 DoubleRow matmuls
     per k-tile into a 2-bank PSUM tile, one ACT exp (the global pacer)
     evacuates both heads to bf16 pt.  Unit order
     (0,0),(0,1),(1,0),(2,0),(3,0),(3,1),(2,1),(1,1) starts on the two
     smallest units (whose projections are resident earliest) and ends
     on a mid-size unit whose own PV chains drain history-first.
  3) Filler work (deferred projections, the previous unit's PV chains)
     is placed by a greedy scheduler that simulates ACT/PE prefix times
     with the instruction cost model and only inserts a filler where it
     cannot stall the exp stream; per-filler DMA-readiness estimates
     and deadline slots (for next unit's diag dependencies) constrain
     placement.
  4) PV in natural layout, f32 PSUM [128, 65] per (head, q-tile); DVE
     reciprocal of the fused denominator column + tensor_scalar_mul.
     Late units stream per-q-tile output DMAs so the tail is short.

Numerics identical to v1: bf16 projection ~2e-4; fp8 q/k quantization
dominates at rel_err ~1.3e-2 (harness gate 2e-2).

_split_matmul_waits() is a required legalization for this compiler
build: every engine instruction may carry at most one semaphore wait.
"""

import numpy as np

import concourse.bass as bass
import concourse.mybir as mybir
import concourse.tile as tile
from concourse.bass_utils import run_bass_kernel_spmd

F32 = mybir.dt.float32
BF = mybir.dt.bfloat16
F8 = mybir.dt.float8e4

FP8_SCORES = True

N, S, D = 2, 2048, 1024
H, HD = 16, 64
P = 128
QB = 512                 # q block (free dim per matmul)
NQB = S // QB            # 4
NKT = S // P             # 16 k tiles
ND = D // P              # 8 contraction tiles
NHC = 4                  # heads per core
EKQ = 2 * NHC * HD       # 512 = k+q rows per core
EV = NHC * HD            # 256 = v rows per core
WS = 16.0                # host W/b pre-scale (fp8 range)
SCALE = 1.0 / 32.0 / (WS * WS)   # 1/sqrt(1024), W-scale compensated

AF = mybir.ActivationFunctionType
ALU = mybir.AluOpType

# ---- cost-model constants for the greedy filler scheduler ----
ACT_C = 0.8333           # ns/elem on ACT
PE_C = 0.4167            # ns/col full rate
SEM = 160.0              # cross-engine handoff guard
GUARD = 120.0


def _exp_ns(w):
    return 2 * w * ACT_C + 185.0


def _mm_tile_ns(w, direct):
    # two matmuls (one per interleaved head) per k-tile
    if not FP8_SCORES:
        return 2 * w * PE_C
    return (2 * w * PE_C) if direct else (w * PE_C)


def _split_matmul_waits(nc):
    """Move extra semaphore waits onto preceding same-engine NOPs.

    The walrus codegen for self-loading matmuls folds waits into the
    LDWEIGHTS struct, which has room for a single sync-wait command;
    sequencer NOPs on the same engine execute in program order, so
    hoisting each wait onto its own NOP is semantics-preserving.
    """
    import bass_rust

    moved = 0
    for bb in nc.main_func.blocks:
        out = []
        for ins in bb.instructions:
            si = ins.sync_info
            keep = 0 if isinstance(ins, bass_rust.InstMatmult) else 1
            if (
                not isinstance(ins, bass_rust.InstNoOp)
                and si is not None
                and len(si.on_wait) > keep
            ):
                hoist = si.on_wait[keep:] if keep else si.on_wait
                for j, w in enumerate(hoist):
                    out.append(
                        bass_rust.InstNoOp(
                            name=f"{ins.name}-hw{j}",
                            engine=ins.engine,
                            sync_info=mybir.SyncInfo(on_wait=[w], on_update=[]),
                        )
                    )
                    moved += 1
                ins.sync_info = mybir.SyncInfo(
                    on_wait=list(si.on_wait[:keep]), on_update=list(si.on_update)
                )
            out.append(ins)
        bb.instructions[:] = out
    return moved


def build_nc(split_waits=True):
    nc = bass.Bass()

    # Contraction rows are host-permuted p-major (row r = p*ND + dt) so
    # every DMA slice below keeps >=512B contiguous runs (the DMA cost
    # model halves bandwidth under 512B).  x and wv pack hi/lo planes
    # with u inner; wkq is split per (head pair, plane) so the startup
    # loads only what the first projection needs.
    x2 = nc.dram_tensor("x2", [2 * D, S], F8, kind="ExternalInput")
    wkqt_d = [[nc.dram_tensor(f"wkq{hp}{u}", [D, 2 * P], F8,
                              kind="ExternalInput")
               for u in range(2)] for hp in range(2)]
    wv2 = nc.dram_tensor("wv2", [2 * D, EV], F8, kind="ExternalInput")
    bkq = nc.dram_tensor("bkq", [P, 4], F32, kind="ExternalInput")
    bv = nc.dram_tensor("bv", [1, EV], F32, kind="ExternalInput")
    o = nc.dram_tensor("o", [S, EV], F32, kind="ExternalOutput")

    x_v = x2.rearrange("(p dt u) s -> p dt u s", p=P, u=2)  # [128,8,2,2048]
    wkq_vs = [[wkqt_d[hp][u].rearrange("(p dt) e -> p dt e", p=P)
               for u in range(2)] for hp in range(2)]       # [128,8,256]
    wv_v = wv2.rearrange("(p dt u) e -> p dt u e", p=P, u=2)  # [128,8,2,256]
    o_v = o.rearrange("(qt p) c -> p qt c", p=P)            # [128, 16, 256]

    with tile.TileContext(nc) as tc:
        with (
            tc.tile_pool(name="const", bufs=1) as const,
            tc.tile_pool(name="big", bufs=1) as big,
            tc.tile_pool(name="xpool", bufs=2) as xpool,
            tc.tile_pool(name="work", bufs=2) as work,
            tc.tile_pool(name="opool", bufs=2) as opool,
            tc.tile_pool(name="proj_ps", bufs=2, space="PSUM") as proj_ps,
            tc.tile_pool(name="st_ps", bufs=2, space="PSUM") as st_ps,
            tc.tile_pool(name="pv_ps", bufs=2, space="PSUM") as pv_ps,
        ):
            # ---- constants ----
            onesb = const.tile([P, 1], BF)
            nc.vector.memset(onesb, 1.0)
            wsb = const.tile([P, 1], BF)
            nc.vector.memset(wsb, WS)
            # warm the ACT exp table while DMAs run
            dummy = const.tile([1, 2], F32)
            nc.gpsimd.memset(dummy, 0.0)
            nc.scalar.activation(dummy, dummy, AF.Exp)
            # diagonal causal mask for the odd head: mask[p, j] = (j >= p)
            mask_sb = const.tile([P, QB], BF)
            nc.gpsimd.affine_select(
                out=mask_sb,
                in_=onesb.to_broadcast((P, QB)),
                compare_op=ALU.is_ge, fill=0.0,
                base=0, channel_multiplier=-1,
                pattern=[[1, QB]],
            )

            # ---- input DMAs, critical-path first ----
            wkqt = [[const.tile([P, ND, 2 * P], F8, name=f"wkq{hp}{u}")
                     for u in range(2)] for hp in range(2)]
            wv_sb = const.tile([P, ND, 2, EV], F8)
            bkq_sb = const.tile([P, 4], F32)
            bvb = const.tile([P, EV], F32)
            xqbs = []
            for qb in range(NQB):
                xq = xpool.tile([P, ND, 2, QB], F8, tag=f"xqb{qb}", bufs=1,
                                name=f"xqb{qb}")
                xqbs.append(xq)

            # DMA cost model: gens serialize on HWDGE (625ns each),
            # transfers serialize on the 16-engine pool at ~360 B/ns,
            # completion semaphores take ~950ns to reach consumers.
            st_dma = {"gen": 1.06e3, "tx": 0.0}

            def dma(out_ap, in_ap, kbytes):
                nc.sync.dma_start(out_ap, in_ap)
                st_dma["gen"] += 625.0
                start = max(st_dma["gen"], st_dma["tx"])
                st_dma["tx"] = start + kbytes * 1024.0 / 360.0
                return st_dma["tx"] + 950.0

            r_bkq = dma(bkq_sb, bkq[:, :], 2)
            r_wkq00 = dma(wkqt[0][0], wkq_vs[0][0], 256)
            r_xq0h = dma(xqbs[0][:, :, 0], x_v[:, :, 0, 0:QB], 512)
            r_xq0l = dma(xqbs[0][:, :, 1], x_v[:, :, 1, 0:QB], 512)
            r_wkq01 = dma(wkqt[0][1], wkq_vs[0][1], 256)
            r_xq1h = dma(xqbs[1][:, :, 0], x_v[:, :, 0, QB:2 * QB], 512)
            r_wv = dma(wv_sb, wv_v[:, :, :, :], 512)
            r_bv = dma(bvb, bv[:, :].partition_broadcast(P), 1)
            r_wkq10 = dma(wkqt[1][0], wkq_vs[1][0], 256)
            r_wkq11 = dma(wkqt[1][1], wkq_vs[1][1], 256)
            r_xq1l = dma(xqbs[1][:, :, 1], x_v[:, :, 1, QB:2 * QB], 512)
            r_xq2 = dma(xqbs[2][:, :, :], x_v[:, :, :, 2 * QB:3 * QB], 1024)
            r_xq3 = dma(xqbs[3][:, :, :], x_v[:, :, :, 3 * QB:4 * QB], 1024)
            x_ready = [(r_xq0h, r_xq0l), (r_xq1h, r_xq1l),
                       (r_xq2, r_xq2), (r_xq3, r_xq3)]
            wkq_ready = [(r_wkq00, r_wkq01), (r_wkq10, r_wkq11)]

            # ---- persistent state ----
            # zkq [p, kq, hp, s]; e-tile t -> (kq=t%2, hp=t//2)
            if FP8_SCORES:
                zkq = big.tile([P, 2, 2, S], F8)
                # DoubleRow re-layout [hl*32+p, kq, hp, g, s], hd = g*32+p
                kq8 = big.tile([HD, 2, 2, 2, S], F8)
            else:
                zkq = big.tile([P, 4, S], BF)
            # v natural + WS column: [p, ktile, head, 65]
            vsb = big.tile([P, NKT, NHC, HD + 1], BF)
            nc.vector.tensor_copy(
                vsb[:, :, :, HD:HD + 1],
                wsb[:, :, None].to_broadcast((P, NKT, NHC, 1)),
            )
            # exp(S^T) per unit: [p, head, ktile, q], double-buffered
            pta = big.tile([P, 2, NKT, QB], BF)
            ptb = big.tile([P, 2, NKT, QB], BF)
            pts = [pta, ptb]

            proj_state = {}
            DR = mybir.MatmulPerfMode.DoubleRow
            # residual passes: z = xh@Wh + xh@Wl + xl@Wh, in an order
            # chosen per granule so the last pass waits on the DMA that
            # lands last for that block.
            PASS_WL = ((0, 0), (1, 0), (0, 1))   # W-lo last (qb0 blocks)
            PASS_XL = ((0, 0), (0, 1), (1, 0))   # x-lo last (qb>=1)

            def proj_kq_half(qb, t, half, passes=PASS_XL, evac_act=False):
                if half == 0:
                    pzp = proj_ps.tile([P, QB], F32, tag="projps",
                                       name=f"pzp{qb}_{t}")
                    proj_state[(qb, t)] = pzp
                else:
                    pzp = proj_state.pop((qb, t)) if half == 2 \
                        else proj_state[(qb, t)]
                xi, wi = passes[half]
                for p2 in range(4):
                    nc.tensor.matmul(
                        pzp,
                        lhsT=wkqt[t // 2][wi][:, 2 * p2:2 * p2 + 2,
                                              (t % 2) * P:(t % 2 + 1) * P],
                        rhs=xqbs[qb][:, 2 * p2:2 * p2 + 2, xi, :],
                        start=(half == 0 and p2 == 0),
                        stop=(half == 2 and p2 == 3),
                        perf_mode=DR,
                    )
                if half < 2:
                    return
                if FP8_SCORES:
                    out = zkq[:, t % 2, t // 2, qb * QB:(qb + 1) * QB]
                else:
                    out = zkq[:, t, qb * QB:(qb + 1) * QB]
                if evac_act:
                    nc.scalar.activation(
                        out, pzp, AF.Identity, bias=bkq_sb[:, t:t + 1]
                    )
                else:
                    nc.vector.tensor_scalar_add(out, pzp, bkq_sb[:, t:t + 1])
                if FP8_SCORES and t % 2 == 1:
                    # both e-tiles of head pair hp = t//2 evacuated:
                    # DoubleRow re-layout via SBUF->SBUF DMAs
                    hp = t // 2
                    qw = slice(qb * QB, (qb + 1) * QB)
                    for hl in range(2):
                        for g in range(2):
                            nc.sync.dma_start(
                                kq8[hl * 32:(hl + 1) * 32, :, hp, g, qw],
                                zkq[hl * HD + g * 32:hl * HD + g * 32 + 32,
                                    :, hp, qw],
                            )

            def proj_v1(qb, qt4):
                qt = qb * 4 + qt4
                pvp = proj_ps.tile([P, QB], F32, tag="projps",
                                   name=f"pvp{qt}")
                for hi in range(3):
                    xi, wi = PASS_XL[hi]
                    for p2 in range(4):
                        nc.tensor.matmul(
                            pvp[:, :EV],
                            lhsT=xqbs[qb][:, 2 * p2:2 * p2 + 2, xi,
                                          qt4 * P:(qt4 + 1) * P],
                            rhs=wv_sb[:, 2 * p2:2 * p2 + 2, wi, :],
                            start=(hi == 0 and p2 == 0),
                            stop=(hi == 2 and p2 == 3),
                            perf_mode=DR,
                        )
                nc.vector.tensor_tensor(
                    vsb[:, qt, :, 0:HD],
                    pvp[:, :EV].rearrange("p (h d) -> p h d", d=HD),
                    bvb.rearrange("p (h d) -> p h d", d=HD),
                    mybir.AluOpType.add,
                )

            def pv_chunk(qb, hp, qt4, hl, osb, pt, hist_first=False,
                         dma_qt=False):
                nkt_q = 4 * qb + qt4 + 1
                if hist_first:
                    kts = list(range(nkt_q))
                else:
                    kts = list(range(4 * qb, nkt_q)) + list(range(0, 4 * qb))
                pvo = pv_ps.tile([P, HD + 1], F32, tag="pv", name="pvo")
                for i, kt in enumerate(kts):
                    nc.tensor.matmul(
                        pvo,
                        lhsT=pt[:, hl, kt, qt4 * P:(qt4 + 1) * P],
                        rhs=vsb[:, kt, 2 * hp + hl, :],
                        start=(i == 0), stop=(i == nkt_q - 1),
                    )
                h = 2 * hp + hl
                rs = work.tile([P, 1], F32, tag="rs", bufs=4)
                nc.vector.reciprocal(rs, pvo[:, HD:HD + 1])
                nc.vector.tensor_scalar_mul(
                    osb[:, qt4, HD * h:HD * (h + 1)],
                    pvo[:, 0:HD], rs,
                )
                if dma_qt and hl == 1:
                    nc.sync.dma_start(
                        o_v[:, qb * 4 + qt4, hp * P:(hp + 1) * P],
                        osb[:, qt4, hp * P:(hp + 1) * P],
                    )

            # ---- greedy filler scheduler state ----
            sched = {"pe": 0.0, "act": 0.0}

            def attn_scores(qb, hp, pt, groups, direct=False,
                            hist_first=False):
                if hist_first:
                    kts = list(range(0, 4 * qb + 4))
                else:
                    kts = list(range(4 * qb, 4 * qb + 4)) + \
                        list(range(4 * qb))
                n = len(kts)
                gq = [list(g) for g in groups]
                exp_end = {}

                def emit(f):
                    f["fn"]()
                    sched["pe"] = max(sched["pe"], f.get("ready", 0.0)) \
                        + f["pe"]

                for i, kt in enumerate(kts):
                    r = kt - 4 * qb
                    diag = 0 <= r < 4
                    off = P * r if diag else 0
                    w = QB - off
                    mmc = _mm_tile_ns(w, direct and diag)
                    # forced fillers (deadline reached)
                    for g in gq:
                        while g and g[0].get("dl", 10 ** 9) <= i:
                            emit(g.pop(0))
                    # opportunistic fillers while ACT has slack
                    progress = True
                    while progress:
                        progress = False
                        for g in gq:
                            if (g and g[0].get("es", -1) <= i
                                    and g[0].get("ready", 0.0) <= sched["pe"]
                                    and sched["pe"] + g[0]["pe"] + mmc
                                    + GUARD <= sched["act"]):
                                emit(g.pop(0))
                                progress = True
                    # st ring WAR: mm_i waits exp_{i-2}
                    if i >= 2:
                        sched["pe"] = max(sched["pe"],
                                          exp_end[i - 2] + 100.0)
                    stp = st_ps.tile([P, 2, QB], F32, tag="st")
                    for hl in range(2):
                        if FP8_SCORES and direct and diag:
                            base = HD * hl
                            nc.tensor.matmul(
                                stp[:, hl, off:QB],
                                lhsT=zkq[base:base + HD, 0, hp,
                                         kt * P:(kt + 1) * P],
                                rhs=zkq[base:base + HD, 1, hp,
                                        qb * QB + off:(qb + 1) * QB],
                                start=True, stop=True,
                            )
                        elif FP8_SCORES:
                            nc.tensor.matmul(
                                stp[:, hl, off:QB],
                                lhsT=kq8[hl * 32:(hl + 1) * 32, 0, hp, :,
                                         kt * P:(kt + 1) * P],
                                rhs=kq8[hl * 32:(hl + 1) * 32, 1, hp, :,
                                        qb * QB + off:(qb + 1) * QB],
                                start=True, stop=True,
                                perf_mode=DR,
                            )
                        else:
                            base = HD * hl
                            nc.tensor.matmul(
                                stp[:, hl, off:QB],
                                lhsT=zkq[base:base + HD, 2 * hp,
                                         kt * P:(kt + 1) * P],
                                rhs=zkq[base:base + HD, 2 * hp + 1,
                                        qb * QB + off:(qb + 1) * QB],
                                start=True, stop=True,
                            )
                    sched["pe"] += mmc
                    est = max(sched["act"], sched["pe"] + SEM)
                    nc.scalar.activation(
                        pt[:, :, kt, off:QB],
                        stp[:, :, off:QB],
                        AF.Exp, scale=SCALE,
                    )
                    sched["act"] = est + _exp_ns(w)
                    exp_end[i] = sched["act"]
                    if diag:
                        nc.gpsimd.affine_select(
                            out=pt[:, 0, kt, off:QB],
                            in_=pt[:, 0, kt, off:QB],
                            compare_op=ALU.is_ge, fill=0.0,
                            base=0, channel_multiplier=-1,
                            pattern=[[1, w]],
                        )
                        nc.vector.tensor_mul(
                            out=pt[:, 1, kt, off:QB],
                            in0=pt[:, 1, kt, off:QB],
                            in1=mask_sb[:, 0:w],
                        )
                # end-of-unit drain
                for g in gq:
                    while g:
                        emit(g.pop(0))

            # ---- filler group constructors ----
            def K(qb2, pair, dl=None):
                passes = PASS_WL if qb2 == 0 else PASS_XL
                tiles = (2 * pair, 2 * pair + 1)
                out = []
                for t in tiles:
                    for h in range(3):
                        xi, wi = passes[h]
                        ready = max(x_ready[qb2][xi], wkq_ready[pair][wi])
                        f = {
                            "fn": (lambda t=t, h=h, q=qb2, ps=passes:
                                   proj_kq_half(q, t, h, passes=ps)),
                            "pe": 427.0, "ready": ready,
                        }
                        if dl is not None:
                            f["dl"] = dl
                        out.append(f)
                return out

            def V(qb2):
                ready = max(x_ready[qb2][1], r_wv, r_bv)
                return [
                    {"fn": (lambda q4=q4, q=qb2: proj_v1(q, q4)),
                     "pe": 640.0, "ready": ready}
                    for q4 in range(4)
                ]

            def PV(pqb, php, ppt, v_qb=None, dma_qt=False):
                # previous unit's PV chains; optionally interleave the
                # V-projection whose vsb tiles those chains read
                out = []
                vg = V(v_qb) if v_qb is not None else []
                for q4 in range(4):
                    if vg:
                        out.append(vg[q4])
                    for hl in range(2):
                        out.append({
                            "fn": (lambda q4=q4, hl=hl:
                                   pv_chunk(pqb, php, q4, hl, osbs[pqb],
                                            ppt, dma_qt=dma_qt)),
                            "pe": (4 * pqb + q4 + 1) * 27.0 + 100.0,
                        })
                return out

            # ---- prologue: pass-major proj of (k,q) head pair 0, qb0 ----
            for half in range(3):
                for t in (0, 1):
                    proj_kq_half(0, t, half, passes=PASS_WL,
                                 evac_act=(t == 1))

            # ---- unit schedule ----
            units = [(0, 0), (0, 1), (1, 0), (2, 0),
                     (3, 0), (3, 1), (2, 1), (1, 1)]
            osbs = {}
            prev = None
            sched["pe"] = 8100.0
            sched["act"] = 8300.0
            for ui, (qb, hp) in enumerate(units):
                pt = pts[ui % 2]
                last = ui == len(units) - 1
                if qb not in osbs:
                    osbs[qb] = opool.tile([P, 4, EV], F32, tag="osb",
                                          bufs=4, name=f"osb{qb}")
                n_slots = 4 * qb + 4
                groups = []
                if prev is not None:
                    pqb, php, ppt = prev
                    # V(j) interleaves with the first PV chains reading it
                    v_qb = {1: 0, 3: 1, 4: 2, 5: 3}.get(ui)
                    groups.append(PV(pqb, php, ppt, v_qb=v_qb,
                                     dma_qt=(ui == 7)))
                if (qb, hp) == (0, 0):
                    groups.append(K(0, 1))
                elif (qb, hp) == (0, 1):
                    groups.append(K(1, 0))
                elif (qb, hp) == (1, 0):
                    groups.append(K(2, 0, dl=n_slots - 2))
                elif (qb, hp) == (2, 0):
                    groups.append(K(3, 0, dl=n_slots - 2))
                    groups.append(K(1, 1))
                elif (qb, hp) == (3, 0):
                    groups.append(K(3, 1, dl=n_slots - 2))
                    groups.append(K(2, 1))
                if last:
                    own = []
                    for q4 in range(4):
                        for hl in range(2):
                            own.append({
                                "fn": (lambda q4=q4, hl=hl:
                                       pv_chunk(qb, hp, q4, hl, osbs[qb],
                                                pt, hist_first=True,
                                                dma_qt=True)),
                                "pe": (4 * qb + q4 + 1) * 27.0 + 100.0,
                                "es": 4 * qb + q4 + 1,
                            })
                    groups.append(own)
                attn_scores(qb, hp, pt, groups,
                            direct=(qb == 0 or (qb, hp) == (1, 0)),
                            hist_first=last)
                if prev is not None and ui != 7:
                    # previous unit's half-row is complete
                    nc.sync.dma_start(
                        o_v[:, pqb * 4:(pqb + 1) * 4,
                            php * P:(php + 1) * P],
                        osbs[pqb][:, :, php * P:(php + 1) * P],
                    )
                prev = (qb, hp, pt)

    if split_waits:
        _split_matmul_waits(nc)
    return nc


_nc_cache = None


def _get_nc():
    global _nc_cache
    if _nc_cache is None:
        _nc_cache = build_nc()
    return _nc_cache


def make_in_maps(x, W, b):
    import ml_dtypes

    f8 = ml_dtypes.float8_e4m3
    # p-major row permutation of the contraction dim: row r = p*ND + dt
    # picks original row dt*128 + p (x and W share it, so z is identical)
    perm = (np.arange(D).reshape(ND, P).T).reshape(-1)

    def hilo(a):
        hi = a.astype(f8)
        lo = (a - hi.astype(np.float32)).astype(f8)
        return hi, lo

    def hilo_pk(a):
        # rows (p dt) -> interleave planes u inner: rows (p dt u)
        hi, lo = hilo(a)
        return np.ascontiguousarray(
            np.stack([hi, lo], axis=1).reshape(2 * a.shape[0], a.shape[1])
        )

    x = np.asarray(x, dtype=np.float32)
    W = np.asarray(W, dtype=np.float32)
    b = np.asarray(b, dtype=np.float32)
    in_maps = []
    x2s = [hilo_pk(x[n].T[perm]) for n in range(N)]
    for c in range(8):
        n, g = divmod(c, 4)
        rk = slice(256 * g, 256 * g + 256)
        rq = slice(D + 256 * g, D + 256 * g + 256)
        rv = slice(2 * D + 256 * g, 2 * D + 256 * g + 256)
        Wk, Wq, Wv = W[rk], W[rq], W[rv]
        m = {"x2": x2s[n],
             "wv2": hilo_pk(Wv.T[perm] * 16.0),
             "bv": np.ascontiguousarray(b[rv].reshape(1, EV) * 16.0)}
        for hp in range(2):
            hr = slice(128 * hp, 128 * hp + 128)
            cols = np.concatenate([Wk[hr], Wq[hr]], axis=0).T[perm] * 16.0
            m[f"wkq{hp}0"], m[f"wkq{hp}1"] = \
                (np.ascontiguousarray(a) for a in hilo(cols))
        bk, bq = b[rk], b[rq]
        m["bkq"] = np.ascontiguousarray(
            np.concatenate([bk[:128], bq[:128], bk[128:], bq[128:]]
                           ).reshape(4, P).T * 16.0
        )
        in_maps.append(m)
    return in_maps


def run(inputs, **kwargs):
    nc = _get_nc()
    in_maps = make_in_maps(inputs["x"], inputs["W"], inputs["b"])
    res = run_bass_kernel_spmd(nc, in_maps, core_ids=list(range(8)), **kwargs)
    out = np.empty((N, S, D), dtype=np.float32)
    for c in range(8):
        n, g = divmod(c, 4)
        out[n, :, 256 * g:256 * g + 256] = res.results[c]["o"]
    return out, res


def kernel(**inputs):
    out, _ = run(inputs)
    return out
